# revision 6
# baseline (speedup 1.0000x reference)
"""Trainium2 Bass kernel for nn_DRO_TOPK (margin-loss top-k + masked sim stats).

Strategy (8 NeuronCores, data-parallel over rows):
  - Each core computes a [512, 4096] slab of sim = X @ X.T via float32r
    matmuls (rhs = full X^T, lhsT = its own 512-row slice of X^T).
  - Per-core inputs are column-ROTATED by c*512 so every core's diagonal
    lands at local cols [0, 512) -> one uniform SPMD program, no per-core
    branching. All reported stats are permutation-invariant.
  - On chip: w = (same_class - 0.5) * -sim ... concretely
    w[i,j] = (notsame[i,j] - 0.5) * sim[i,j]  in {-s/2, +s/2}, so that
    pair_loss = relu(MARGIN + 2*w) and top-k(pair_loss) = relu-map of
    top-k(w). Device emits per-row top-8 of w (max8) plus bracketed
    zero-loss counts. Everything else (mean_pos/mean_neg/pos-neg counts)
    is computed on host in f64 -- mathematically exact.
  - Host merges the 8 cores' candidates, proves per-row-top-8 sufficiency,
    and falls back to a full numpy recompute if any guard fails.
"""

import os
import sys

import numpy as np

for _p in ('/opt/trn_rl_repo', '/root/.axon_site/_ro/trn_rl_repo'):
    if os.path.isdir(_p) and _p not in sys.path:
        sys.path.insert(0, _p)

N, D, NCORES = 4096, 512, 8
R = N // NCORES            # 512 rows per core
NT = R // 128              # 4 row-tiles per core
CH = N // 512              # 8 column chunks of 512
KK = D // 128              # 4 contraction sub-tiles
MARGIN, BETA, TOPK = 0.5, 0.0, 20
ZTHR = -MARGIN / 2.0       # w <= ZTHR  <=>  pair_loss == 0
DELTA = 1e-3               # zero-count bracket width

_prog_cache = {}


def _round_f32r(a):
    """Round f32 array to float32r (RN to 11 mantissa bits), so the on-device
    f32r matmul consumes exactly these values via a non-casting DMA."""
    bits = a.astype(np.float32).view(np.uint32)
    rnd = ((bits.astype(np.uint64) + 0x800) >> 12 << 12).astype(np.uint32)
    return rnd.view(np.float32)


def _build_program():
    import concourse.bacc as bacc
    import concourse.mybir as mybir
    from concourse.tile import TileContext

    f32 = mybir.dt.float32
    f32r = mybir.dt.float32r
    Alu = mybir.AluOpType

    nc = bacc.Bacc('TRN2', target_bir_lowering=False, debug=False)
    xtr_d = nc.dram_tensor('xtr', [D, N], f32r, kind='ExternalInput')
    tgr_d = nc.dram_tensor('tgr', [N], f32, kind='ExternalInput')
    cand_d = nc.dram_tensor('cand', [R, 8], f32, kind='ExternalOutput')
    zlo_d = nc.dram_tensor('zlo', [R], f32, kind='ExternalOutput')
    zhi_d = nc.dram_tensor('zhi', [R], f32, kind='ExternalOutput')

    with TileContext(nc) as tc:
        with (
            tc.tile_pool(name='xts', bufs=1) as xts_pool,
            tc.tile_pool(name='tb', bufs=1) as tb_pool,
            tc.tile_pool(name='w', bufs=2) as w_pool,
            tc.tile_pool(name='mb', bufs=2) as mb_pool,
            tc.tile_pool(name='zs', bufs=1) as zs_pool,
            tc.tile_pool(name='small', bufs=1) as small_pool,
            tc.tile_pool(name='ps', bufs=6, space='PSUM') as ps_pool,
        ):
            # X^T (rotated) in SBUF: 4 partition-tiles of [128, 4096] f32r.
            xts = [xts_pool.tile([128, CH, 512], f32r, tag=f'xt{kk}',
                                 name=f'xts{kk}') for kk in range(KK)]
            xtr_v = xtr_d[:, :].rearrange('(a p) (c f) -> a p c f', p=128, f=512)
            for kk in range(KK):
                for k in range(CH):
                    nc.sync.dma_start(xts[kk][:, k, :], xtr_v[kk, :, k, :])

            # Rotated targets broadcast to all 128 partitions.
            tb = tb_pool.tile([128, N], f32)
            nc.sync.dma_start(tb[:, :], tgr_d[:].unsqueeze(0).partition_broadcast(128))
            # Per-partition row targets: tr[p, t] = tgr[t*128 + p].
            tr = small_pool.tile([128, NT], f32, tag='tr')
            nc.sync.dma_start(tr[:, :], tgr_d[0:R].rearrange('(t p) -> p t', p=128))

            cand_sb = small_pool.tile([128, NT, 8], f32, tag='cand')
            bias_hi = small_pool.tile([128, 1], f32, tag='bias_hi')
            nc.vector.memset(bias_hi[:, :], -(ZTHR + DELTA))
            zacc_lo = small_pool.tile([128, NT], f32, tag='zlo')
            zacc_hi = small_pool.tile([128, NT], f32, tag='zhi')

            for t in range(NT):
                # notsame mask for this row-tile over all 4096 cols (one op).
                mb = mb_pool.tile([128, N], f32)
                nc.vector.tensor_scalar(mb[:, :], tb[:, :], tr[:, t:t + 1], None,
                                        Alu.not_equal)
                w = w_pool.tile([128, N], f32)
                for k in range(CH):
                    ps = ps_pool.tile([128, 512], f32)
                    for kk in range(KK):
                        nc.tensor.matmul(ps[:, :],
                                         xts[kk][:, 0, t * 128:(t + 1) * 128],
                                         xts[kk][:, k, :],
                                         start=(kk == 0), stop=(kk == KK - 1))
                    # w = (notsame - 0.5) * sim
                    nc.vector.scalar_tensor_tensor(
                        w[:, k * 512:(k + 1) * 512],
                        mb[:, k * 512:(k + 1) * 512], 0.5, ps[:, :],
                        Alu.subtract, Alu.mult)
                # Per-row top-8 candidates of w.
                nc.vector.max(cand_sb[:, t, :], w[:, :])
                # Bracketed zero-loss counts (diag rows included; host corrects).
                z1 = zs_pool.tile([128, N], f32, tag='z1')
                nc.scalar.activation(z1[:, :], w[:, :],
                                     mybir.ActivationFunctionType.Sign,
                                     bias=bias_hi[:, :],
                                     accum_out=zacc_hi[:, t:t + 1])
                z2 = zs_pool.tile([128, N], f32, tag='z2')
                nc.vector.tensor_scalar(z2[:, :], w[:, :], ZTHR - DELTA, 1.0,
                                        Alu.is_le, Alu.mult,
                                        accum_out=zacc_lo[:, t:t + 1])

            nc.sync.dma_start(cand_d[:, :].rearrange('(t p) j -> p t j', p=128),
                              cand_sb[:, :, :])
            nc.sync.dma_start(zlo_d[:].rearrange('(t p) -> p t', p=128),
                              zacc_lo[:, :])
            nc.sync.dma_start(zhi_d[:].rearrange('(t p) -> p t', p=128),
                              zacc_hi[:, :])

    nc.compile()
    return nc


def _numpy_fallback(x, t):
    """Faithful f32 numpy recompute of the full reference (safety net)."""
    sim = x @ x.T
    same = t[:, None] == t[None, :]
    eye = np.eye(N, dtype=bool)
    pos = same & ~eye
    neg = ~same
    pos_l = np.maximum(MARGIN + BETA - sim, 0.0).astype(np.float32)
    neg_l = np.maximum(MARGIN + sim - BETA, 0.0).astype(np.float32)
    valid = pos | neg
    pair = np.where(pos, pos_l, neg_l)
    zeros = int((valid & (pair == 0.0)).sum())
    masked = np.where(valid, pair, -np.inf).ravel()
    top = np.sort(masked)[-TOPK:]
    loss = np.float32(top.astype(np.float64).mean())
    mean_pos = np.float32(sim[pos].astype(np.float64).sum() / pos.sum())
    mean_neg = np.float32(sim[neg].astype(np.float64).sum() / neg.sum())
    return loss, np.int32(zeros), mean_pos, mean_neg


def kernel(**inputs):
    from concourse.bass_utils import run_bass_kernel_spmd

    x = np.ascontiguousarray(inputs['inputs'].astype(np.float32, copy=False))
    t = np.asarray(inputs['targets'])
    t_i = t.astype(np.int64)
    t_f = t.astype(np.float32)

    if 'nc' not in _prog_cache:
        _prog_cache['nc'] = _build_program()
    nc = _prog_cache['nc']

    xt = _round_f32r(np.ascontiguousarray(x.T))          # [D, N] f32r values
    in_maps = []
    for c in range(NCORES):
        sh = c * R
        xtr_c = np.ascontiguousarray(np.roll(xt, -sh, axis=1))
        tgr_c = np.ascontiguousarray(np.roll(t_f, -sh))
        in_maps.append({'xtr': xtr_c, 'tgr': tgr_c})

    res = run_bass_kernel_spmd(nc, in_maps, core_ids=list(range(NCORES)))

    cand = np.concatenate([r['cand'] for r in res.results], axis=0)  # [N, 8]
    zlo = np.concatenate([r['zlo'] for r in res.results])            # [N]
    zhi = np.concatenate([r['zhi'] for r in res.results])            # [N]

    # ---- top-k loss from device candidates ----
    flat = cand.ravel()
    top_all = np.sort(flat)[-TOPK:]
    t20 = top_all[0]
    sufficiency_ok = bool((cand[:, 7] <= t20).all())

    # ---- zero count guards ----
    # zlo: exact count of (w <= ZTHR - DELTA) per row (incl. the diag element).
    # zhi: sum of sign(w - (ZTHR + DELTA)) per row -> count_le = (N - zhi)/2.
    c_lo = zlo
    c_hi = (N - zhi) / 2.0
    zeros_ok = (np.all(c_lo == np.round(c_lo)) and np.all(c_hi == np.round(c_hi))
                and np.all(c_lo == c_hi))
    num_zeros = int(c_lo.sum()) - N  # remove the N diagonal elements

    if not (sufficiency_ok and zeros_ok):
        return _numpy_fallback(x, t_i)

    loss = np.float32(np.maximum(MARGIN + 2.0 * top_all.astype(np.float64), 0.0).mean())

    # ---- exact f64 stats on host ----
    x64 = x.astype(np.float64)
    G = np.zeros((int(t_i.max()) + 1, D), dtype=np.float64)
    np.add.at(G, t_i, x64)
    cls_sq = float((G * G).sum())
    diag_sq = float((x64 * x64).sum())
    cnt = np.bincount(t_i)
    pos_cnt = int((cnt.astype(np.int64) * (cnt - 1)).sum())
    neg_cnt = N * N - int((cnt.astype(np.int64) ** 2).sum())
    tot = x64.sum(axis=0)
    total_sq = float(tot @ tot)
    mean_pos = np.float32((cls_sq - diag_sq) / pos_cnt)
    mean_neg = np.float32((total_sq - cls_sq) / neg_cnt)

    return loss, np.int32(num_zeros), mean_pos, mean_neg


# revision 8
# speedup vs baseline: 1.4333x; 1.4333x over previous
"""Trainium2 Bass kernel for nn_DRO_TOPK (margin-loss top-k + masked sim stats).

Strategy (8 NeuronCores, data-parallel over rows):
  - Each core computes a [512, 4096] slab of sim = X @ X.T via float32r
    matmuls (rhs = full X^T, lhsT = its own 512-row slice of X^T).
  - Per-core inputs are column-ROTATED by c*512 so every core's diagonal
    lands at local cols [0, 512) -> one uniform SPMD program, no per-core
    branching. All reported stats are permutation-invariant.
  - On chip: w[i,j] = (notsame[i,j] - 0.5) * sim[i,j] in {-s/2, +s/2}, so
    pair_loss = relu(MARGIN + 2*w) and top-k(pair_loss) = relu-map of
    top-k(w). Device emits per-row top-8 of w (max8) plus Sign-bracketed
    zero-loss counts on the Scalar engine. Everything else
    (mean_pos/mean_neg/counts) is computed on host in f64 -- exact.
  - Host merges the 8 cores' candidates, proves per-row-top-8 sufficiency,
    and falls back to a full numpy recompute if any guard fails.
"""

import os
import sys

import numpy as np

for _p in ('/opt/trn_rl_repo', '/root/.axon_site/_ro/trn_rl_repo'):
    if os.path.isdir(_p) and _p not in sys.path:
        sys.path.insert(0, _p)

N, D, NCORES = 4096, 512, 8
R = N // NCORES            # 512 rows per core
NT = R // 128              # 4 row-tiles per core
CH = N // 512              # 8 column chunks of 512
CP = N // 1024             # 4 column chunk-pairs (1024 wide, 2 PSUM banks)
KK = D // 128              # 4 contraction sub-tiles
MARGIN, BETA, TOPK = 0.5, 0.0, 20
ZTHR = -MARGIN / 2.0       # w <= ZTHR  <=>  pair_loss == 0
DELTA = 1e-3               # zero-count bracket width

_prog_cache = {}


def _round_f32r(a):
    """Round f32 array to float32r (RN to 11 mantissa bits), so the on-device
    f32r matmul consumes exactly these values via a non-casting DMA."""
    bits = a.astype(np.float32).view(np.uint32)
    rnd = ((bits.astype(np.uint64) + 0x800) >> 12 << 12).astype(np.uint32)
    return rnd.view(np.float32)


def _build_program():
    import concourse.bacc as bacc
    import concourse.mybir as mybir
    from concourse.tile import TileContext

    f32 = mybir.dt.float32
    f16 = mybir.dt.float16
    f32r = mybir.dt.float32r
    Alu = mybir.AluOpType
    Act = mybir.ActivationFunctionType

    nc = bacc.Bacc('TRN2', target_bir_lowering=False, debug=False)
    xtr_d = nc.dram_tensor('xtr', [D, N], f32r, kind='ExternalInput')
    tgr_d = nc.dram_tensor('tgr', [N], f16, kind='ExternalInput')
    tgf_d = nc.dram_tensor('tgf', [R], f32, kind='ExternalInput')
    cand_d = nc.dram_tensor('cand', [R, 8], f32, kind='ExternalOutput')
    zlo_d = nc.dram_tensor('zlo', [R, 2], f32, kind='ExternalOutput')
    zhi_d = nc.dram_tensor('zhi', [R, 2], f32, kind='ExternalOutput')

    with TileContext(nc) as tc:
        with (
            tc.tile_pool(name='xts', bufs=1) as xts_pool,
            tc.tile_pool(name='tb', bufs=1) as tb_pool,
            tc.tile_pool(name='w', bufs=2) as w_pool,
            tc.tile_pool(name='mb', bufs=2) as mb_pool,
            tc.tile_pool(name='zs', bufs=1) as zs_pool,
            tc.tile_pool(name='small', bufs=1) as small_pool,
            tc.tile_pool(name='ps', bufs=4, space='PSUM') as ps_pool,
        ):
            # X^T (rotated) in SBUF: 4 partition-tiles of [128, 4096] f32r.
            xts = [xts_pool.tile([128, CH, 512], f32r, tag=f'xt{kk}',
                                 name=f'xts{kk}') for kk in range(KK)]
            xtr_v = xtr_d[:, :].rearrange('(a p) (c f) -> a p c f', p=128, f=512)
            # k-major issue order: chunk-0 pieces land first so the PE can
            # start after ~1MB instead of the full 8MB.
            for k in range(CH):
                for kk in range(KK):
                    nc.sync.dma_start(xts[kk][:, k, :], xtr_v[kk, :, k, :])

            # Rotated targets (f16) broadcast to all 128 partitions.
            tb = tb_pool.tile([128, N], f16)
            nc.sync.dma_start(tb[:, :], tgr_d[:].unsqueeze(0).partition_broadcast(128))
            # Per-partition row targets: tr[p, t] = tgr[t*128 + p].
            tr = small_pool.tile([128, NT], f32, tag='tr')
            nc.sync.dma_start(tr[:, :], tgf_d[:].rearrange('(t p) -> p t', p=128))

            cand_sb = small_pool.tile([128, NT, 8], f32, tag='cand')
            zacc_lo = small_pool.tile([128, NT, 2], f32, tag='zlo')
            zacc_hi = small_pool.tile([128, NT, 2], f32, tag='zhi')
            bias_hi = small_pool.tile([128, 1], f32, tag='bias_hi')
            nc.vector.memset(bias_hi[:, :], -(ZTHR + DELTA))
            bias_lo = small_pool.tile([128, 1], f32, tag='bias_lo')
            nc.vector.memset(bias_lo[:, :], -(ZTHR - DELTA))

            for t in range(NT):
                # (notsame - 0.5) in fp16 for this row-tile, all 4096 cols.
                mb = mb_pool.tile([128, N], f16)
                nc.vector.tensor_scalar(mb[:, :], tb[:, :], tr[:, t:t + 1], 0.5,
                                        Alu.not_equal, Alu.subtract)
                w = w_pool.tile([128, N], f32)
                for cp in range(CP):
                    ps = ps_pool.tile([128, 1024], f32)
                    for h in range(2):
                        k = cp * 2 + h
                        for kk in range(KK):
                            nc.tensor.matmul(ps[:, h * 512:(h + 1) * 512],
                                             xts[kk][:, 0, t * 128:(t + 1) * 128],
                                             xts[kk][:, k, :],
                                             start=(kk == 0), stop=(kk == KK - 1))
                    # w = (notsame - 0.5) * sim over a 2-bank span
                    nc.vector.tensor_tensor(
                        w[:, cp * 1024:(cp + 1) * 1024],
                        mb[:, cp * 1024:(cp + 1) * 1024], ps[:, :],
                        op=Alu.mult)
                # Per-row top-8 candidates of w.
                nc.vector.max(cand_sb[:, t, :], w[:, :])
                # Sign-bracketed zero-loss counts on ACT, split in halves.
                for hv in range(2):
                    wh = w[:, hv * 2048:(hv + 1) * 2048]
                    z1 = zs_pool.tile([128, 2048], f32, tag='z1')
                    nc.scalar.activation(z1[:, :], wh, Act.Sign,
                                         bias=bias_hi[:, :],
                                         accum_out=zacc_hi[:, t, hv:hv + 1])
                    z2 = zs_pool.tile([128, 2048], f32, tag='z2')
                    nc.scalar.activation(z2[:, :], wh, Act.Sign,
                                         bias=bias_lo[:, :],
                                         accum_out=zacc_lo[:, t, hv:hv + 1])

            nc.sync.dma_start(cand_d[:, :].rearrange('(t p) j -> p t j', p=128),
                              cand_sb[:, :, :])
            nc.sync.dma_start(zlo_d[:, :].rearrange('(t p) h -> p t h', p=128),
                              zacc_lo[:, :, :])
            nc.sync.dma_start(zhi_d[:, :].rearrange('(t p) h -> p t h', p=128),
                              zacc_hi[:, :, :])

    nc.compile()
    return nc


def _numpy_fallback(x, t):
    """Faithful f32 numpy recompute of the full reference (safety net)."""
    sim = x @ x.T
    same = t[:, None] == t[None, :]
    eye = np.eye(N, dtype=bool)
    pos = same & ~eye
    neg = ~same
    pos_l = np.maximum(MARGIN + BETA - sim, 0.0).astype(np.float32)
    neg_l = np.maximum(MARGIN + sim - BETA, 0.0).astype(np.float32)
    valid = pos | neg
    pair = np.where(pos, pos_l, neg_l)
    zeros = int((valid & (pair == 0.0)).sum())
    masked = np.where(valid, pair, -np.inf).ravel()
    top = np.sort(masked)[-TOPK:]
    loss = np.float32(top.astype(np.float64).mean())
    mean_pos = np.float32(sim[pos].astype(np.float64).sum() / pos.sum())
    mean_neg = np.float32(sim[neg].astype(np.float64).sum() / neg.sum())
    return loss, np.int32(zeros), mean_pos, mean_neg


def kernel(**inputs):
    from concourse.bass_utils import run_bass_kernel_spmd

    x = np.ascontiguousarray(inputs['inputs'].astype(np.float32, copy=False))
    t = np.asarray(inputs['targets'])
    t_i = t.astype(np.int64)
    t_f = t.astype(np.float16)
    t_f32 = t.astype(np.float32)

    if 'nc' not in _prog_cache:
        _prog_cache['nc'] = _build_program()
    nc = _prog_cache['nc']

    xt = _round_f32r(np.ascontiguousarray(x.T))          # [D, N] f32r values
    in_maps = []
    for c in range(NCORES):
        sh = c * R
        xtr_c = np.ascontiguousarray(np.roll(xt, -sh, axis=1))
        tgr_c = np.ascontiguousarray(np.roll(t_f, -sh))
        tgf_c = np.ascontiguousarray(np.roll(t_f32, -sh)[:R])
        in_maps.append({'xtr': xtr_c, 'tgr': tgr_c, 'tgf': tgf_c})

    res = run_bass_kernel_spmd(nc, in_maps, core_ids=list(range(NCORES)))

    cand = np.concatenate([r['cand'] for r in res.results], axis=0)  # [N, 8]
    zsum_lo = np.concatenate([r['zlo'] for r in res.results]).sum(axis=1)  # [N]
    zsum_hi = np.concatenate([r['zhi'] for r in res.results]).sum(axis=1)  # [N]

    # ---- top-k loss from device candidates ----
    flat = cand.ravel()
    top_all = np.sort(flat)[-TOPK:]
    t20 = top_all[0]
    sufficiency_ok = bool((cand[:, 7] <= t20).all())

    # ---- zero count guards ----
    c_lo = (N - zsum_lo) / 2.0   # count of w <= ZTHR - DELTA (incl. diag)
    c_hi = (N - zsum_hi) / 2.0   # count of w <= ZTHR + DELTA (incl. diag)
    zeros_ok = (np.all(c_lo == np.round(c_lo)) and np.all(c_hi == np.round(c_hi))
                and np.all(c_lo == c_hi))
    num_zeros = int(c_lo.sum()) - N  # remove the N diagonal elements

    if not (sufficiency_ok and zeros_ok):
        return _numpy_fallback(x, t_i)

    loss = np.float32(np.maximum(MARGIN + 2.0 * top_all.astype(np.float64), 0.0).mean())

    # ---- exact f64 stats on host ----
    x64 = x.astype(np.float64)
    G = np.zeros((int(t_i.max()) + 1, D), dtype=np.float64)
    np.add.at(G, t_i, x64)
    cls_sq = float((G * G).sum())
    diag_sq = float((x64 * x64).sum())
    cnt = np.bincount(t_i)
    pos_cnt = int((cnt.astype(np.int64) * (cnt - 1)).sum())
    neg_cnt = N * N - int((cnt.astype(np.int64) ** 2).sum())
    tot = x64.sum(axis=0)
    total_sq = float(tot @ tot)
    mean_pos = np.float32((cls_sq - diag_sq) / pos_cnt)
    mean_neg = np.float32((total_sq - cls_sq) / neg_cnt)

    return loss, np.int32(num_zeros), mean_pos, mean_neg


# revision 9
# speedup vs baseline: 1.5302x; 1.0676x over previous
"""Trainium2 Bass kernel for nn_DRO_TOPK (margin-loss top-k + masked sim stats).

Strategy (8 NeuronCores, data-parallel over rows, symmetry-halved):
  - sim = X @ X.T is symmetric: every unordered pair {i, j} is covered once
    by the half-circle band d = (j - i) mod 4096 in [1, 2048]. Each core
    computes, for its 512 rows, a [128, 2176]-wide rectangle per row-tile
    (cols [a, a+2176) in core-local rotated coordinates, a = t*128) that
    covers each row's band plus <=128 junk cells/row (diag + mirror
    duplicates), which the host filters out by index.
  - Per-core inputs are column-ROTATED by c*512 so the band always sits at
    local cols [0, 2560) -> one uniform SPMD program; only 5 of 8 MB of
    X^T per core is ever touched.
  - On chip: w[i,j] = (notsame - 0.5) * sim in {-s/2, +s/2}; pair_loss =
    relu(MARGIN + 2*w), monotone in w. Device emits per-row top-8 of w
    (max8 + max_index) and Sign-bracketed zero-loss counts on the Scalar
    engine. Matmuls run as float32r (1 cyc/row at N>=256).
  - Host: drops junk by index (d outside [1,2048]), recomputes surviving
    candidate sims exactly in f64, takes top-10 unique pairs (x2 = the
    reference's top-20), and computes mean_pos/mean_neg/counts exactly in
    f64. Guards (top-8 sufficiency, zero-count bracket) trigger a full
    numpy fallback if the fast path cannot be proven exact.
"""

import os
import sys

import numpy as np

for _p in ('/opt/trn_rl_repo', '/root/.axon_site/_ro/trn_rl_repo'):
    if os.path.isdir(_p) and _p not in sys.path:
        sys.path.insert(0, _p)

N, D, NCORES = 4096, 512, 8
R = N // NCORES            # 512 rows per core
NT = R // 128              # 4 row-tiles per core
HB = N // 2                # 2048 half-circle band width
W_RECT = HB + 128          # 2176 rect width per row-tile
XCOLS = 3 * 128 + W_RECT   # 2560 cols of rotated X^T each core touches
KK = D // 128              # 4 contraction sub-tiles
MARGIN, BETA, TOPK = 0.5, 0.0, 20
ZTHR = -MARGIN / 2.0       # w <= ZTHR  <=>  pair_loss == 0
DELTA = 1e-3               # zero-count bracket width

_prog_cache = {}


def _round_f32r(a):
    """Round f32 array to float32r (RN to 11 mantissa bits), so the on-device
    f32r matmul consumes exactly these values via a non-casting DMA."""
    bits = a.astype(np.float32).view(np.uint32)
    rnd = ((bits.astype(np.uint64) + 0x800) >> 12 << 12).astype(np.uint32)
    return rnd.view(np.float32)


def _build_program():
    import concourse.bacc as bacc
    import concourse.mybir as mybir
    from concourse.tile import TileContext

    f32 = mybir.dt.float32
    f16 = mybir.dt.float16
    u32 = mybir.dt.uint32
    f32r = mybir.dt.float32r
    Alu = mybir.AluOpType
    Act = mybir.ActivationFunctionType

    nc = bacc.Bacc('TRN2', target_bir_lowering=False, debug=False)
    xtr_d = nc.dram_tensor('xtr', [D, XCOLS], f32r, kind='ExternalInput')
    tgr_d = nc.dram_tensor('tgr', [XCOLS], f16, kind='ExternalInput')
    tgf_d = nc.dram_tensor('tgf', [R], f32, kind='ExternalInput')
    cand_d = nc.dram_tensor('cand', [R, 8], f32, kind='ExternalOutput')
    candi_d = nc.dram_tensor('candi', [R, 8], u32, kind='ExternalOutput')
    zlo_d = nc.dram_tensor('zlo', [R], f32, kind='ExternalOutput')
    zhi_d = nc.dram_tensor('zhi', [R], f32, kind='ExternalOutput')

    with TileContext(nc) as tc:
        with (
            tc.tile_pool(name='xts', bufs=1) as xts_pool,
            tc.tile_pool(name='tb', bufs=1) as tb_pool,
            tc.tile_pool(name='w', bufs=2) as w_pool,
            tc.tile_pool(name='mb', bufs=2) as mb_pool,
            tc.tile_pool(name='zs', bufs=1) as zs_pool,
            tc.tile_pool(name='small', bufs=1) as small_pool,
            tc.tile_pool(name='psb', bufs=3, space='PSUM') as psb_pool,
            tc.tile_pool(name='pst', bufs=2, space='PSUM') as pst_pool,
        ):
            # Rotated X^T in SBUF: 4 partition-tiles of [128, 2560] f32r.
            xts = [xts_pool.tile([128, XCOLS], f32r, tag=f'xt{kk}',
                                 name=f'xts{kk}') for kk in range(KK)]
            xtr_v = xtr_d[:, :].rearrange('(a p) c -> a p c', p=128)
            # k-major issue order: low columns land first so the PE can start
            # after ~1MB instead of the full 5MB.
            for k in range(XCOLS // 512):
                for kk in range(KK):
                    nc.sync.dma_start(xts[kk][:, k * 512:(k + 1) * 512],
                                      xtr_v[kk, :, k * 512:(k + 1) * 512])

            # Rotated targets (f16) broadcast to all 128 partitions.
            tb = tb_pool.tile([128, XCOLS], f16)
            nc.sync.dma_start(tb[:, :], tgr_d[:].unsqueeze(0).partition_broadcast(128))
            # Per-partition row targets (f32): tr[p, t] = target[t*128 + p].
            tr = small_pool.tile([128, NT], f32, tag='tr')
            nc.sync.dma_start(tr[:, :], tgf_d[:].rearrange('(t p) -> p t', p=128))

            cand_sb = small_pool.tile([128, NT, 8], f32, tag='cand')
            candi_sb = small_pool.tile([128, NT, 8], u32, tag='candi')
            zacc_lo = small_pool.tile([128, NT], f32, tag='zlo')
            zacc_hi = small_pool.tile([128, NT], f32, tag='zhi')
            bias_hi = small_pool.tile([128, 1], f32, tag='bias_hi')
            nc.vector.memset(bias_hi[:, :], -(ZTHR + DELTA))
            bias_lo = small_pool.tile([128, 1], f32, tag='bias_lo')
            nc.vector.memset(bias_lo[:, :], -(ZTHR - DELTA))

            for t in range(NT):
                a = t * 128
                # (notsame - 0.5) in fp16 for this row-tile's rect.
                mb = mb_pool.tile([128, W_RECT], f16)
                nc.vector.tensor_scalar(mb[:, :], tb[:, a:a + W_RECT],
                                        tr[:, t:t + 1], 0.5,
                                        Alu.not_equal, Alu.subtract)
                w = w_pool.tile([128, W_RECT], f32)
                # band pieces: 2x [128,1024] (2 PSUM banks) + 1x [128,128]
                for piece in range(2):
                    ps = psb_pool.tile([128, 1024], f32, name=f'psb{t}_{piece}',
                                       tag='psb')
                    for h in range(2):
                        o = a + piece * 1024 + h * 512
                        for kk in range(KK):
                            nc.tensor.matmul(ps[:, h * 512:(h + 1) * 512],
                                             xts[kk][:, a:a + 128],
                                             xts[kk][:, o:o + 512],
                                             start=(kk == 0), stop=(kk == KK - 1))
                    nc.vector.tensor_tensor(
                        w[:, piece * 1024:(piece + 1) * 1024],
                        mb[:, piece * 1024:(piece + 1) * 1024], ps[:, :],
                        op=Alu.mult)
                pt = pst_pool.tile([128, 128], f32, tag='pst')
                o = a + 2048
                for kk in range(KK):
                    nc.tensor.matmul(pt[:, :], xts[kk][:, a:a + 128],
                                     xts[kk][:, o:o + 128],
                                     start=(kk == 0), stop=(kk == KK - 1))
                nc.vector.tensor_tensor(w[:, 2048:2176], mb[:, 2048:2176],
                                        pt[:, :], op=Alu.mult)
                # Per-row top-8 candidates of w (+ their rect indices).
                nc.vector.max(cand_sb[:, t, :], w[:, :])
                nc.vector.max_index(candi_sb[:, t, :], cand_sb[:, t, :], w[:, :])
                # Sign-bracketed zero-loss counts on ACT.
                z1 = zs_pool.tile([128, W_RECT], f32, tag='z1')
                nc.scalar.activation(z1[:, :], w[:, :], Act.Sign,
                                     bias=bias_hi[:, :],
                                     accum_out=zacc_hi[:, t:t + 1])
                z2 = zs_pool.tile([128, W_RECT], f32, tag='z2')
                nc.scalar.activation(z2[:, :], w[:, :], Act.Sign,
                                     bias=bias_lo[:, :],
                                     accum_out=zacc_lo[:, t:t + 1])

            nc.sync.dma_start(cand_d[:, :].rearrange('(t p) j -> p t j', p=128),
                              cand_sb[:, :, :])
            nc.sync.dma_start(candi_d[:, :].rearrange('(t p) j -> p t j', p=128),
                              candi_sb[:, :, :])
            nc.sync.dma_start(zlo_d[:].rearrange('(t p) -> p t', p=128),
                              zacc_lo[:, :])
            nc.sync.dma_start(zhi_d[:].rearrange('(t p) -> p t', p=128),
                              zacc_hi[:, :])

    nc.compile()
    return nc


def _numpy_fallback(x, t):
    """Faithful f32 numpy recompute of the full reference (safety net)."""
    sim = x @ x.T
    same = t[:, None] == t[None, :]
    eye = np.eye(N, dtype=bool)
    pos = same & ~eye
    neg = ~same
    pos_l = np.maximum(MARGIN + BETA - sim, 0.0).astype(np.float32)
    neg_l = np.maximum(MARGIN + sim - BETA, 0.0).astype(np.float32)
    valid = pos | neg
    pair = np.where(pos, pos_l, neg_l)
    zeros = int((valid & (pair == 0.0)).sum())
    masked = np.where(valid, pair, -np.inf).ravel()
    top = np.sort(masked)[-TOPK:]
    loss = np.float32(top.astype(np.float64).mean())
    mean_pos = np.float32(sim[pos].astype(np.float64).sum() / pos.sum())
    mean_neg = np.float32(sim[neg].astype(np.float64).sum() / neg.sum())
    return loss, np.int32(zeros), mean_pos, mean_neg


def kernel(**inputs):
    from concourse.bass_utils import run_bass_kernel_spmd

    x = np.ascontiguousarray(inputs['inputs'].astype(np.float32, copy=False))
    t = np.asarray(inputs['targets'])
    t_i = t.astype(np.int64)
    t16 = t.astype(np.float16)
    t32 = t.astype(np.float32)

    if 'nc' not in _prog_cache:
        _prog_cache['nc'] = _build_program()
    nc = _prog_cache['nc']

    xt = _round_f32r(np.ascontiguousarray(x.T))          # [D, N] f32r values
    xt2 = np.concatenate([xt, xt[:, :XCOLS - N]], axis=1)   # wrap for rotation
    t16w = np.concatenate([t16, t16[:XCOLS - N]])
    in_maps = []
    for c in range(NCORES):
        sh = c * R
        in_maps.append({
            'xtr': np.ascontiguousarray(xt2[:, sh:sh + XCOLS]),
            'tgr': np.ascontiguousarray(t16w[sh:sh + XCOLS]),
            'tgf': np.ascontiguousarray(t32[sh:sh + R]),
        })

    res = run_bass_kernel_spmd(nc, in_maps, core_ids=list(range(NCORES)))

    cand = np.concatenate([r['cand'] for r in res.results], axis=0)   # [N, 8]
    candi = np.concatenate([r['candi'] for r in res.results], axis=0)
    zsum_lo = np.concatenate([r['zlo'] for r in res.results])         # [N]
    zsum_hi = np.concatenate([r['zhi'] for r in res.results])

    # ---- index bookkeeping: row r of cand is global row (r), candidate x
    # refers to core-local col a + x, global col (c*512 + a + x) mod N,
    # where a = (r % 512) // 128 * 128. d = x - p, p = r % 128. ----
    rows = np.arange(N)
    p = rows % 128
    d = candi - p[:, None]                       # pair distance, [N, 8]
    gcol = (rows[:, None] - p[:, None] + candi) % N
    keep = (d >= 1) & ((d <= HB - 1) | ((d == HB) & (rows[:, None] < gcol)))

    # ---- exact f64 values for kept candidates ----
    x64 = x.astype(np.float64)
    ri, ci = np.nonzero(keep)
    gi = rows[ri]
    gj = gcol[ri, ci]
    s_exact = np.einsum('nd,nd->n', x64[gi], x64[gj])
    sign = np.where(t_i[gi] == t_i[gj], -1.0, 1.0)
    w_exact = 0.5 * sign * s_exact

    # ---- top-10 unique pairs (x2 = reference top-20) ----
    order = np.argsort(w_exact)[::-1]
    top10 = w_exact[order[:TOPK // 2]]
    t10_dev = np.sort(cand[keep].ravel())[-(TOPK // 2)]
    sufficiency_ok = (len(w_exact) >= TOPK // 2 and
                      bool((cand[:, 7] <= t10_dev).all()))

    # ---- zero count guards (expect exactly the diag junk cell per row) ----
    c_lo = (W_RECT - zsum_lo) / 2.0
    c_hi = (W_RECT - zsum_hi) / 2.0
    zeros_ok = (np.all(c_lo == 1.0) and np.all(c_hi == 1.0))
    if not (sufficiency_ok and zeros_ok):
        return _numpy_fallback(x, t_i)
    num_zeros = 0

    top20 = np.repeat(top10, 2)
    loss = np.float32(np.maximum(MARGIN + 2.0 * top20, 0.0).mean())

    # ---- exact f64 stats on host ----
    G = np.zeros((int(t_i.max()) + 1, D), dtype=np.float64)
    np.add.at(G, t_i, x64)
    cls_sq = float((G * G).sum())
    diag_sq = float((x64 * x64).sum())
    cnt = np.bincount(t_i)
    pos_cnt = int((cnt.astype(np.int64) * (cnt - 1)).sum())
    neg_cnt = N * N - int((cnt.astype(np.int64) ** 2).sum())
    tot = x64.sum(axis=0)
    total_sq = float(tot @ tot)
    mean_pos = np.float32((cls_sq - diag_sq) / pos_cnt)
    mean_neg = np.float32((total_sq - cls_sq) / neg_cnt)

    return loss, np.int32(num_zeros), mean_pos, mean_neg


# revision 10
# speedup vs baseline: 2.0408x; 1.3336x over previous
"""Trainium2 Bass kernel for nn_DRO_TOPK (margin-loss top-k + masked sim stats).

Strategy (8 NeuronCores, data-parallel over rows, symmetry-halved):
  - sim = X @ X.T is symmetric: every unordered pair {i, j} is covered once
    by the half-circle band d = (j - i) mod 4096 in [1, 2048]. Each core
    computes, for its 512 rows, a [128, 2176]-wide rectangle per row-tile
    (cols [a, a+2176) in core-local rotated coordinates, a = t*128) that
    covers each row's band plus <=128 junk cells/row (diag + mirror
    duplicates), which the host filters out by index.
  - Per-core inputs are column-ROTATED by c*512 so the band always sits at
    local cols [0, 2560) -> one uniform SPMD program; only 5 of 8 MB of
    X^T per core is ever touched.
  - On chip: w[i,j] = (notsame - 0.5) * sim in {-s/2, +s/2}; pair_loss =
    relu(MARGIN + 2*w), monotone in w. Device emits per-row top-8 of w
    (max8 + max_index) and Sign-bracketed zero-loss counts on the Scalar
    engine. Matmuls run as float32r (1 cyc/row at N>=256).
  - Host: drops junk by index (d outside [1,2048]), recomputes surviving
    candidate sims exactly in f64, takes top-10 unique pairs (x2 = the
    reference's top-20), and computes mean_pos/mean_neg/counts exactly in
    f64. Guards (top-8 sufficiency, zero-count bracket) trigger a full
    numpy fallback if the fast path cannot be proven exact.
"""

import os
import sys

import numpy as np

for _p in ('/opt/trn_rl_repo', '/root/.axon_site/_ro/trn_rl_repo'):
    if os.path.isdir(_p) and _p not in sys.path:
        sys.path.insert(0, _p)

N, D, NCORES = 4096, 512, 8
R = N // NCORES            # 512 rows per core
NT = R // 128              # 4 row-tiles per core
HB = N // 2                # 2048 half-circle band width
W_RECT = HB + 128          # 2176 rect width per row-tile
XCOLS = 3 * 128 + W_RECT   # 2560 cols of rotated X^T each core touches
KK = D // 128              # 4 contraction sub-tiles
MARGIN, BETA, TOPK = 0.5, 0.0, 20
ZTHR = -MARGIN / 2.0       # w <= ZTHR  <=>  pair_loss == 0
DELTA = 1e-3               # zero-count bracket width

_prog_cache = {}


def _round_f32r(a):
    """Round f32 array to float32r (RN to 11 mantissa bits), so the on-device
    f32r matmul consumes exactly these values via a non-casting DMA."""
    bits = a.astype(np.float32).view(np.uint32)
    rnd = ((bits.astype(np.uint64) + 0x800) >> 12 << 12).astype(np.uint32)
    return rnd.view(np.float32)


def _build_program():
    import concourse.bacc as bacc
    import concourse.mybir as mybir
    from concourse.tile import TileContext

    f32 = mybir.dt.float32
    f16 = mybir.dt.float16
    u32 = mybir.dt.uint32
    f32r = mybir.dt.float32r
    Alu = mybir.AluOpType
    Act = mybir.ActivationFunctionType

    nc = bacc.Bacc('TRN2', target_bir_lowering=False, debug=False)
    xtr_d = nc.dram_tensor('xtr', [KK, 128, XCOLS], f32r, kind='ExternalInput')
    tgr_d = nc.dram_tensor('tgr', [XCOLS], f16, kind='ExternalInput')
    tgf_d = nc.dram_tensor('tgf', [128, NT], f32, kind='ExternalInput')
    # fused output, device-native layout: per partition p:
    # [cand(t,j): 32 | candi(t,j) u32-bits: 32 | zlo(t): 4 | zhi(t): 4]
    outp_d = nc.dram_tensor('outp', [128, 72], f32, kind='ExternalOutput')

    with TileContext(nc) as tc:
        with (
            tc.tile_pool(name='xts', bufs=1) as xts_pool,
            tc.tile_pool(name='tb', bufs=1) as tb_pool,
            tc.tile_pool(name='w', bufs=2) as w_pool,
            tc.tile_pool(name='mb', bufs=2) as mb_pool,
            tc.tile_pool(name='zs', bufs=1) as zs_pool,
            tc.tile_pool(name='small', bufs=1) as small_pool,
            tc.tile_pool(name='psb', bufs=3, space='PSUM') as psb_pool,
            tc.tile_pool(name='pst', bufs=2, space='PSUM') as pst_pool,
        ):
            # Rotated X^T in SBUF: 4 partition-tiles of [128, 2560] f32r,
            # each a single contiguous-per-partition DMA (fat descriptors).
            xts = [xts_pool.tile([128, XCOLS], f32r, tag=f'xt{kk}',
                                 name=f'xts{kk}') for kk in range(KK)]
            for kk in range(KK):
                nc.sync.dma_start(xts[kk][:, :], xtr_d[kk, :, :])

            # Rotated targets (f16) broadcast to all 128 partitions.
            tb = tb_pool.tile([128, XCOLS], f16)
            nc.sync.dma_start(tb[:, :], tgr_d[:].unsqueeze(0).partition_broadcast(128))
            # Per-partition row targets (f32): tr[p, t] = target[t*128 + p].
            tr = small_pool.tile([128, NT], f32, tag='tr')
            nc.sync.dma_start(tr[:, :], tgf_d[:, :])

            outt = small_pool.tile([128, 72], f32, tag='outt')
            bias_hi = small_pool.tile([128, 1], f32, tag='bias_hi')
            nc.vector.memset(bias_hi[:, :], -(ZTHR + DELTA))
            bias_lo = small_pool.tile([128, 1], f32, tag='bias_lo')
            nc.vector.memset(bias_lo[:, :], -(ZTHR - DELTA))

            for t in range(NT):
                a = t * 128
                # (notsame - 0.5) in fp16 for this row-tile's rect.
                mb = mb_pool.tile([128, W_RECT], f16)
                nc.vector.tensor_scalar(mb[:, :], tb[:, a:a + W_RECT],
                                        tr[:, t:t + 1], 0.5,
                                        Alu.not_equal, Alu.subtract)
                w = w_pool.tile([128, W_RECT], f32)
                # band pieces: 2x [128,1024] (2 PSUM banks) + 1x [128,128]
                for piece in range(2):
                    ps = psb_pool.tile([128, 1024], f32, name=f'psb{t}_{piece}',
                                       tag='psb')
                    for h in range(2):
                        o = a + piece * 1024 + h * 512
                        for kk in range(KK):
                            nc.tensor.matmul(ps[:, h * 512:(h + 1) * 512],
                                             xts[kk][:, a:a + 128],
                                             xts[kk][:, o:o + 512],
                                             start=(kk == 0), stop=(kk == KK - 1))
                    nc.vector.tensor_tensor(
                        w[:, piece * 1024:(piece + 1) * 1024],
                        mb[:, piece * 1024:(piece + 1) * 1024], ps[:, :],
                        op=Alu.mult)
                pt = pst_pool.tile([128, 128], f32, tag='pst')
                o = a + 2048
                for kk in range(KK):
                    nc.tensor.matmul(pt[:, :], xts[kk][:, a:a + 128],
                                     xts[kk][:, o:o + 128],
                                     start=(kk == 0), stop=(kk == KK - 1))
                nc.vector.tensor_tensor(w[:, 2048:2176], mb[:, 2048:2176],
                                        pt[:, :], op=Alu.mult)
                # Per-row top-8 candidates of w (+ their rect indices).
                nc.vector.max(outt[:, t * 8:(t + 1) * 8], w[:, :])
                nc.vector.max_index(
                    outt[:, 32 + t * 8:32 + (t + 1) * 8].bitcast(u32),
                    outt[:, t * 8:(t + 1) * 8], w[:, :])
                # Sign-bracketed zero-loss counts on ACT.
                z1 = zs_pool.tile([128, W_RECT], f32, tag='z1')
                nc.scalar.activation(z1[:, :], w[:, :], Act.Sign,
                                     bias=bias_hi[:, :],
                                     accum_out=outt[:, 68 + t:69 + t])
                z2 = zs_pool.tile([128, W_RECT], f32, tag='z2')
                nc.scalar.activation(z2[:, :], w[:, :], Act.Sign,
                                     bias=bias_lo[:, :],
                                     accum_out=outt[:, 64 + t:65 + t])

            nc.sync.dma_start(outp_d[:, :], outt[:, :])

    nc.compile()
    return nc


def _numpy_fallback(x, t):
    """Faithful f32 numpy recompute of the full reference (safety net)."""
    sim = x @ x.T
    same = t[:, None] == t[None, :]
    eye = np.eye(N, dtype=bool)
    pos = same & ~eye
    neg = ~same
    pos_l = np.maximum(MARGIN + BETA - sim, 0.0).astype(np.float32)
    neg_l = np.maximum(MARGIN + sim - BETA, 0.0).astype(np.float32)
    valid = pos | neg
    pair = np.where(pos, pos_l, neg_l)
    zeros = int((valid & (pair == 0.0)).sum())
    masked = np.where(valid, pair, -np.inf).ravel()
    top = np.sort(masked)[-TOPK:]
    loss = np.float32(top.astype(np.float64).mean())
    mean_pos = np.float32(sim[pos].astype(np.float64).sum() / pos.sum())
    mean_neg = np.float32(sim[neg].astype(np.float64).sum() / neg.sum())
    return loss, np.int32(zeros), mean_pos, mean_neg


def kernel(**inputs):
    from concourse.bass_utils import run_bass_kernel_spmd

    x = np.ascontiguousarray(inputs['inputs'].astype(np.float32, copy=False))
    t = np.asarray(inputs['targets'])
    t_i = t.astype(np.int64)
    t16 = t.astype(np.float16)
    t32 = t.astype(np.float32)

    if 'nc' not in _prog_cache:
        _prog_cache['nc'] = _build_program()
    nc = _prog_cache['nc']

    xt = _round_f32r(np.ascontiguousarray(x.T))          # [D, N] f32r values
    xt2 = np.concatenate([xt, xt[:, :XCOLS - N]], axis=1)   # wrap for rotation
    t16w = np.concatenate([t16, t16[:XCOLS - N]])
    in_maps = []
    for c in range(NCORES):
        sh = c * R
        in_maps.append({
            'xtr': np.ascontiguousarray(
                xt2[:, sh:sh + XCOLS].reshape(KK, 128, XCOLS)),
            'tgr': np.ascontiguousarray(t16w[sh:sh + XCOLS]),
            'tgf': np.ascontiguousarray(
                t32[sh:sh + R].reshape(NT, 128).T),
        })

    res = run_bass_kernel_spmd(nc, in_maps, core_ids=list(range(NCORES)))

    cands, candis, zlos, zhis = [], [], [], []
    for r in res.results:
        o = r['outp']                                   # [128, 72]
        cands.append(o[:, 0:32].reshape(128, NT, 8).transpose(1, 0, 2)
                     .reshape(R, 8))
        candis.append(o[:, 32:64].view(np.uint32).reshape(128, NT, 8)
                      .transpose(1, 0, 2).reshape(R, 8))
        zlos.append(o[:, 64:68].T.reshape(R))
        zhis.append(o[:, 68:72].T.reshape(R))
    cand = np.concatenate(cands, axis=0)                # [N, 8]
    candi = np.concatenate(candis, axis=0).astype(np.int64)
    zsum_lo = np.concatenate(zlos)                      # [N]
    zsum_hi = np.concatenate(zhis)

    # ---- index bookkeeping: row r of cand is global row (r), candidate x
    # refers to core-local col a + x, global col (c*512 + a + x) mod N,
    # where a = (r % 512) // 128 * 128. d = x - p, p = r % 128. ----
    rows = np.arange(N)
    p = rows % 128
    d = candi - p[:, None]                       # pair distance, [N, 8]
    gcol = (rows[:, None] - p[:, None] + candi) % N
    keep = (d >= 1) & ((d <= HB - 1) | ((d == HB) & (rows[:, None] < gcol)))

    # ---- exact f64 values for kept candidates ----
    x64 = x.astype(np.float64)
    ri, ci = np.nonzero(keep)
    gi = rows[ri]
    gj = gcol[ri, ci]
    s_exact = np.einsum('nd,nd->n', x64[gi], x64[gj])
    sign = np.where(t_i[gi] == t_i[gj], -1.0, 1.0)
    w_exact = 0.5 * sign * s_exact

    # ---- top-10 unique pairs (x2 = reference top-20) ----
    order = np.argsort(w_exact)[::-1]
    top10 = w_exact[order[:TOPK // 2]]
    t10_dev = np.sort(cand[keep].ravel())[-(TOPK // 2)]
    sufficiency_ok = (len(w_exact) >= TOPK // 2 and
                      bool((cand[:, 7] <= t10_dev).all()))

    # ---- zero count guards (expect exactly the diag junk cell per row) ----
    c_lo = (W_RECT - zsum_lo) / 2.0
    c_hi = (W_RECT - zsum_hi) / 2.0
    zeros_ok = (np.all(c_lo == 1.0) and np.all(c_hi == 1.0))
    if not (sufficiency_ok and zeros_ok):
        return _numpy_fallback(x, t_i)
    num_zeros = 0

    top20 = np.repeat(top10, 2)
    loss = np.float32(np.maximum(MARGIN + 2.0 * top20, 0.0).mean())

    # ---- exact f64 stats on host ----
    G = np.zeros((int(t_i.max()) + 1, D), dtype=np.float64)
    np.add.at(G, t_i, x64)
    cls_sq = float((G * G).sum())
    diag_sq = float((x64 * x64).sum())
    cnt = np.bincount(t_i)
    pos_cnt = int((cnt.astype(np.int64) * (cnt - 1)).sum())
    neg_cnt = N * N - int((cnt.astype(np.int64) ** 2).sum())
    tot = x64.sum(axis=0)
    total_sq = float(tot @ tot)
    mean_pos = np.float32((cls_sq - diag_sq) / pos_cnt)
    mean_neg = np.float32((total_sq - cls_sq) / neg_cnt)

    return loss, np.int32(num_zeros), mean_pos, mean_neg
